# revision 27
# baseline (speedup 1.0000x reference)
"""Dense GAT (2-layer, 8+1 heads) on 8 Trainium2 NeuronCores — V3.

Row-parallel over destination rows (R=512 per core). Key algebra: softmax
normalization cancels any pure-i factor, so with s = a_src[j] + a_dst[i]:

    exp(lrelu(s))/exp(.2 ad[i]) = max(exp(as[j])*exp(.8 ad[i]), exp(.2 as[j]))

Per (head, j-tile) the unmasked weights are ONE 4x-mode DVE op:
    m = tensor_scalar(u8bc, scalar1=eas[j], scalar2=e2as[j], mult, max)
then masked with a quad-batched tensor_tensor against adjT. A fraction of
quads runs on ACT (Relu with AP scale/bias) + Pool (tensor_mul) to use all
three elementwise engines. adjT comes pre-transposed from the host; row
broadcasts go through a DRAM bounce (DMA, no engine time).
"""
import numpy as np

N = 4096
F_IN = 256
HID = 64
H1 = 8
F1 = H1 * HID
OUT = 128
N_CORES = 8
R = N // N_CORES
JT = N // 128
IT = R // 128
NQ = JT // 4           # jt quads per head
G2 = 68                # gather cols: 64 f32 = h2|ones fp16-packed, eas2, e2as2, ne2as2
NEG_ATT = 0.2
NEG_OUT = 0.01

# Quad engine assignment (Bresenham-interleaved fractions):
#   A = DVE op1 + DVE mask-mult
#   B = ACT(Relu)+ACT(Identity) + DVE mask-mult
#   D = DVE op1 + Pool mask-mult
QFRAC = {"B": 5, "D": 9, "A": 10}   # per 24 quads

_CACHE = {}
_QTYPES = []
_acc = {k: 0 for k in QFRAC}
for _g in range(80):
    for k in QFRAC:
        _acc[k] += QFRAC[k]
    _pick = max(_acc, key=lambda k: _acc[k])
    _acc[_pick] -= 24
    _QTYPES.append(_pick)


L2QTYPES = "ABADADBA"


def _qtype(g):
    if g >= H1 * NQ:
        return L2QTYPES[g - H1 * NQ]
    return _QTYPES[g]


def _build():
    import concourse.bass as bass
    from concourse import bacc
    import concourse.mybir as mybir
    import concourse.tile as tile
    from concourse.masks import make_identity

    f32 = mybir.dt.float32
    f16 = mybir.dt.float16
    A = mybir.ActivationFunctionType
    Al = mybir.AluOpType

    nc = bacc.Bacc("TRN2", target_bir_lowering=False, debug=False,
                   num_devices=N_CORES)
    d_xT = nc.dram_tensor("xT", [F_IN, N], f16, kind="ExternalInput")
    d_adjT = nc.dram_tensor("adjT", [N, R], f16, kind="ExternalInput")
    d_rhs1 = nc.dram_tensor("rhs1", [F_IN, F1 + H1], f16, kind="ExternalInput")
    d_vdst1 = nc.dram_tensor("vdst1", [F_IN, H1], f16, kind="ExternalInput")
    d_rhs2 = nc.dram_tensor("rhs2", [F1, OUT + 2], f16, kind="ExternalInput")
    d_b1c = nc.dram_tensor("b1c", [HID, H1], f32, kind="ExternalInput")
    d_b2c = nc.dram_tensor("b2c", [OUT, 1], f32, kind="ExternalInput")
    d_outT = nc.dram_tensor("outT", [OUT, R], f32, kind="ExternalOutput")
    c_lo = 0  # this core's own-row block: cols c*R..(c+1)*R of xT handled via xmT input
    d_xmT = nc.dram_tensor("xmT", [F_IN, R], f16, kind="ExternalInput")

    with tile.TileContext(nc) as tc, \
         nc.allow_low_precision(reason="fp16 softmax weights; tolerance 2e-2"):
        with tc.tile_pool(name="const", bufs=1) as const, \
             tc.tile_pool(name="big", bufs=1) as big, \
             tc.tile_pool(name="work", bufs=3) as work, \
             tc.tile_pool(name="dram", bufs=1, space="DRAM") as dram, \
             tc.tile_pool(name="ps_mm", bufs=2, space="PSUM") as ps_mm, \
             tc.tile_pool(name="ps_h2p", bufs=1, space="PSUM") as ps_h2p, \
             tc.tile_pool(name="ps_z2p", bufs=1, space="PSUM") as ps_z2p, \
             tc.tile_pool(name="ps_sm", bufs=2, space="PSUM") as ps_sm, \
             tc.tile_pool(name="ps_ag", bufs=2, space="PSUM") as ps_ag:
            ident = const.tile([128, 128], f32)
            make_identity(nc, ident)
            ones16 = const.tile([128, 1], f16)
            nc.vector.memset(ones16, 1.0)
            rhs1_sb = const.tile([128, 2, F1 + H1], f16)
            nc.sync.dma_start(
                out=rhs1_sb,
                in_=d_rhs1[:, :].rearrange("(kb p) f -> p kb f", p=128))
            vdst1_sb = const.tile([128, 2, H1], f16)
            nc.sync.dma_start(
                out=vdst1_sb,
                in_=d_vdst1[:, :].rearrange("(kb p) f -> p kb f", p=128))
            rhs2_sb = const.tile([128, 4, OUT + 2], f16)
            nc.sync.dma_start(
                out=rhs2_sb,
                in_=d_rhs2[:, :].rearrange("(kt p) f -> p kt f", p=128))
            b1_sb = const.tile([HID, H1], f32)
            nc.sync.dma_start(out=b1_sb, in_=d_b1c[:, :])
            b2_sb = const.tile([OUT, 1], f32)
            nc.sync.dma_start(out=b2_sb, in_=d_b2c[:, :])

            adjT_all = big.tile([128, JT, R], f16)          # 32 KB/part
            h1_all = big.tile([128, JT, H1, HID + 1], f16)  # 32.5 KB/part
            asrc_all = big.tile([128, JT, H1], f32)
            eas_all = big.tile([128, JT, H1], f32)
            e2as_all = big.tile([128, JT, H1], f32)
            ne2as_all = big.tile([128, JT, H1], f32)
            u8bc_all = big.tile([128, H1, R], f16)          # 8 KB/part
            u8bc2 = big.tile([128, R], f16)
            x2T_all = big.tile([128, 4, R], f16)
            adstT = big.tile([H1, R], f32)
            adst2T = big.tile([1, R], f32)
            h2g_all = big.tile([128, 2, N_CORES, 2, G2], f32)  # 17.4 KB/part

            nc.vector.memset(h1_all[:, :, :, HID:HID + 1], 1.0)

            # ---- own rows first (tiny), then interleaved adjT/xT chunks ----
            xm16 = big.tile([128, 2, R], f16)
            nc.sync.dma_start(
                out=xm16, in_=d_xmT[:, :].rearrange("(kb p) r -> p kb r", p=128))
            xt_all = big.tile([128, 2, N], f16)           # 16 KB/part
            for b in range(4):
                nc.sync.dma_start(
                    out=adjT_all[:, b * 8:(b + 1) * 8, :],
                    in_=d_adjT[b * 1024:(b + 1) * 1024, :].rearrange(
                        "(jt p) r -> p jt r", p=128))
                cols = slice(b * 1024, (b + 1) * 1024)
                for kb in range(2):
                    nc.sync.dma_start(out=xt_all[:, kb, cols],
                                      in_=d_xT[kb * 128:(kb + 1) * 128, cols])

            # ---- a_dst (own rows) + u8 broadcasts ----
            for it in range(IT):
                ps_ad = ps_sm.tile([128, H1], f32, tag="sm")
                for kb in range(2):
                    nc.tensor.matmul(ps_ad,
                                     xm16[:, kb, it * 128:(it + 1) * 128],
                                     vdst1_sb[:, kb, :],
                                     start=(kb == 0), stop=(kb == 1))
                adm = work.tile([128, H1], f32, tag="adm", bufs=2)
                nc.vector.tensor_copy(adm, ps_ad)
                ps_adT = ps_sm.tile([H1, 128], f32, tag="sm")
                nc.tensor.transpose(ps_adT, adm, ident)
                nc.vector.tensor_copy(adstT[:, it * 128:(it + 1) * 128], ps_adT)
            u8rows = work.tile([H1, R], f16, tag="u8r", bufs=1)
            nc.scalar.activation(u8rows, adstT, A.Exp, scale=1.0 - NEG_ATT)
            d_u8 = dram.tile([H1, R], f16, name="d_u8")
            nc.sync.dma_start(out=d_u8, in_=u8rows)
            nc.sync.dma_start(out=u8bc_all[:, None, :, :],
                              in_=d_u8[:, :].partition_broadcast(128))

            # ---- h1 | a_src ----
            for jt in range(JT):
                cols = slice(jt * 128, (jt + 1) * 128)
                ps_h = ps_mm.tile([128, F1], f32, tag="h")
                ps_a = ps_sm.tile([128, H1], f32, tag="sm")
                for kb in range(2):
                    nc.tensor.matmul(ps_h, xt_all[:, kb, cols],
                                     rhs1_sb[:, kb, 0:F1],
                                     start=(kb == 0), stop=(kb == 1))
                    nc.tensor.matmul(ps_a, xt_all[:, kb, cols],
                                     rhs1_sb[:, kb, F1:F1 + H1],
                                     start=(kb == 0), stop=(kb == 1))
                h1dst = h1_all[:, jt, :, 0:HID]
                h1src = ps_h.rearrange("p (h c) -> p h c", c=HID)
                if jt % 4 == 0 and jt < 16:
                    nc.vector.tensor_copy(h1dst, h1src)
                else:
                    nc.scalar.copy(h1dst, h1src)
                nc.scalar.copy(asrc_all[:, jt, :], ps_a)
                if jt % 8 == 7:
                    g = slice(jt - 7, jt + 1)
                    nc.scalar.activation(eas_all[:, g, :],
                                         asrc_all[:, g, :], A.Exp)
                    nc.scalar.activation(e2as_all[:, g, :],
                                         asrc_all[:, g, :], A.Exp,
                                         scale=NEG_ATT)
                    nc.vector.tensor_scalar_mul(ne2as_all[:, g, :],
                                                e2as_all[:, g, :], -1.0)

            ps_h2m = ps_h2p.tile([128, 4, OUT], f32, name="ph2m")  # 1 bank

            # PE pacing: tiny matmuls dependent on freshly produced data keep
            # the tensor engine's p-state run alive through producer stalls.
            def pe_tick(src_slice):
                ps_d = ps_sm.tile([64, 64], f32, tag="sm")
                nc.tensor.matmul(ps_d, src_slice, src_slice[:, 0:64],
                                 start=True, stop=True)

            # ---- layer-1 attention ----
            bounce_in = [dram.tile([R // 2, G2], f32, name=f"bin{hf}")
                         for hf in range(2)]
            bounce_out = [dram.tile([N_CORES, R // 2, G2], f32,
                                    addr_space="Shared", name=f"bout{hf}")
                          for hf in range(2)]
            for h in range(H1):
                ps_agg = ps_ag.tile([HID + 1, R], f32, tag="agg")
                for q in range(NQ):
                    qt = _qtype(h * NQ + q)
                    jts = [q * 4 + k for k in range(4)]
                    adjq = adjT_all[:, q * 4:q * 4 + 4, :]
                    e_q = work.tile([128, 4, R], f16, tag="eq", bufs=5)
                    m_q = work.tile([128, 4, R], f16, tag="mq", bufs=5)
                    if qt == "B":
                        for k, jt in enumerate(jts):
                            t_q = work.tile([128, R], f16, tag="tq", bufs=4)
                            nc.scalar.activation(
                                t_q, u8bc_all[:, h, :], A.Relu,
                                bias=ne2as_all[:, jt, h:h + 1],
                                scale=eas_all[:, jt, h:h + 1])
                            nc.scalar.activation(
                                m_q[:, k, :], t_q, A.Identity,
                                bias=e2as_all[:, jt, h:h + 1])
                            pe_tick(m_q[:, k, 0:64])
                    else:
                        for k, jt in enumerate(jts):
                            nc.vector.tensor_scalar(
                                m_q[:, k, :], u8bc_all[:, h, :],
                                eas_all[:, jt, h:h + 1],
                                e2as_all[:, jt, h:h + 1],
                                op0=Al.mult, op1=Al.max)
                            pe_tick(m_q[:, k, 0:64])
                    if qt == "D":
                        nc.gpsimd.tensor_mul(e_q, m_q, adjq)
                    elif qt == "B":
                        for z in range(2):
                            nc.vector.tensor_tensor(
                                e_q[:, z * 2:z * 2 + 2, :],
                                m_q[:, z * 2:z * 2 + 2, :],
                                adjq[:, z * 2:z * 2 + 2, :], op=Al.mult)
                    else:
                        nc.vector.tensor_tensor(e_q, m_q, adjq, op=Al.mult)
                    for k, jt in enumerate(jts):
                        nc.tensor.matmul(ps_agg, h1_all[:, jt, h, :],
                                         e_q[:, k, :],
                                         start=(jt == 0), stop=(jt == JT - 1))
                # fast-release: one ACT copy frees ps_agg, then normalize
                # runs on SBUF fp16 off the critical path
                cp = work.tile([HID + 1, R], f16, tag="cp", bufs=3)
                nc.vector.tensor_copy(cp, ps_agg)
                rz = work.tile([1, R], f16, tag="rz", bufs=2)
                nc.vector.reciprocal(rz, cp[HID:HID + 1, :])
                d_rz = dram.tile([1, R], f16, name=f"d_rz{h}")
                nc.sync.dma_start(out=d_rz, in_=rz)
                rzb = work.tile([HID, R], f16, tag="rzb", bufs=2)
                nc.sync.dma_start(out=rzb[:, None, :],
                                  in_=d_rz[0:1, :].partition_broadcast(HID))
                y_h = work.tile([HID, R], f16, tag="yh", bufs=2)
                nc.vector.tensor_tensor(y_h, cp[0:HID, :], rzb, op=Al.mult)
                po = (h % 2) * HID
                nc.scalar.activation(
                    x2T_all[po:po + HID, h // 2, :], y_h, A.Prelu,
                    bias=b1_sb[:, h:h + 1], alpha=NEG_OUT)

            # ---- layer 2: pack h2 + gather cols, single AllGather ----
            for it in range(IT):
                cols = slice(it * 128, (it + 1) * 128)
                ps_a2 = ps_sm.tile([128, 2], f32, tag="sm")
                for kt in range(4):
                    nc.tensor.matmul(ps_a2, x2T_all[:, kt, cols],
                                     rhs2_sb[:, kt, OUT:OUT + 2],
                                     start=(kt == 0), stop=(kt == 3))
                for kt in range(4):
                    nc.tensor.matmul(ps_h2m[:, it, :],
                                     x2T_all[:, kt, cols],
                                     rhs2_sb[:, kt, 0:OUT],
                                     start=(kt == 0), stop=(kt == 3))
                h2m = work.tile([128, G2], f32, tag="h2m", bufs=2)
                h2m16 = h2m.bitcast(f16)
                nc.scalar.copy(h2m16[:, 0:OUT], ps_h2m[:, it, :])
                nc.vector.memset(h2m16[:, OUT:OUT + 2], 1.0)
                nc.scalar.activation(h2m[:, 65:66], ps_a2[:, 0:1], A.Exp)
                nc.scalar.activation(h2m[:, 66:67], ps_a2[:, 0:1],
                                     A.Exp, scale=NEG_ATT)
                nc.vector.tensor_scalar_mul(h2m[:, 67:68], h2m[:, 66:67], -1.0)
                nc.sync.dma_start(
                    out=bounce_in[it // 2][(it % 2) * 128:(it % 2 + 1) * 128, :],
                    in_=h2m)
                ad2c = work.tile([128, 1], f32, tag="ad2c", bufs=2)
                nc.vector.tensor_copy(ad2c, ps_a2[:, 1:2])
                ps_adT2 = ps_sm.tile([1, 128], f32, tag="sm")
                nc.tensor.transpose(ps_adT2, ad2c, ident)
                nc.vector.tensor_copy(adst2T[:, it * 128:(it + 1) * 128],
                                      ps_adT2)
            for hf in range(2):
                nc.gpsimd.collective_compute(
                    "AllGather",
                    bass.mybir.AluOpType.bypass,
                    replica_groups=[list(range(N_CORES))],
                    ins=[bounce_in[hf].opt()],
                    outs=[bounce_out[hf].opt()],
                )
            u8row2 = work.tile([1, R], f16, tag="u8r2", bufs=1)
            nc.scalar.activation(u8row2, adst2T, A.Exp, scale=1.0 - NEG_ATT)
            d_u82 = dram.tile([1, R], f16, name="d_u82")
            nc.sync.dma_start(out=d_u82, in_=u8row2)
            nc.sync.dma_start(out=u8bc2[:, None, :],
                              in_=d_u82[0:1, :].partition_broadcast(128))
            for hf in range(2):
                for c4 in range(N_CORES):
                    nc.sync.dma_start(
                        out=h2g_all[:, hf, c4, :, :],
                        in_=bounce_out[hf][c4].rearrange(
                            "(r p) g -> p r g", p=128))

            # ---- layer-2 attention ----
            ps_o2 = ps_ag.tile([128, R], f32, tag="agg")
            ps_z2 = ps_z2p.tile([1, R], f32, name="z2")
            L2P = "ADBADADA"  # per (half, c4-pairstep) engine types

            def slot(jt):
                return h2g_all[:, (jt % 4) // 2, jt // 4, jt % 2, :]

            step = 0
            for hf in range(2):
                for c4 in range(N_CORES):
                    jts = [c4 * 4 + hf * 2 + r2 for r2 in range(2)]
                    qt = L2P[(hf * N_CORES + c4) % len(L2P)]
                    adjp = adjT_all[:, jts[0]:jts[0] + 2, :]
                    e_q = work.tile([128, 2, R], f16, tag="eq2", bufs=5)
                    m_q = work.tile([128, 2, R], f16, tag="mq2", bufs=5)
                    if qt == "B":
                        for k, jt in enumerate(jts):
                            h2g = slot(jt)
                            t_q = work.tile([128, R], f16, tag="tq", bufs=4)
                            nc.scalar.activation(
                                t_q, u8bc2, A.Relu,
                                bias=h2g[:, 67:68], scale=h2g[:, 65:66])
                            nc.scalar.activation(
                                m_q[:, k, :], t_q, A.Identity,
                                bias=h2g[:, 66:67])
                    else:
                        for k, jt in enumerate(jts):
                            h2g = slot(jt)
                            nc.vector.tensor_scalar(
                                m_q[:, k, :], u8bc2,
                                h2g[:, 65:66], h2g[:, 66:67],
                                op0=Al.mult, op1=Al.max)
                            pe_tick(m_q[:, k, 0:64])
                    if qt == "D":
                        nc.gpsimd.tensor_mul(e_q, m_q, adjp)
                    else:
                        nc.vector.tensor_tensor(e_q, m_q, adjp, op=Al.mult)
                    for k, jt in enumerate(jts):
                        h2g = slot(jt)
                        h2g16 = h2g[:, 0:OUT // 2].bitcast(f16)
                        ones16 = h2g[:, OUT // 2:OUT // 2 + 1].bitcast(f16)[:, 0:1]
                        nc.tensor.matmul(ps_o2, h2g16, e_q[:, k, :],
                                         start=(step == 0), stop=(step == JT - 1))
                        nc.tensor.matmul(ps_z2, ones16, e_q[:, k, :],
                                         start=(step == 0), stop=(step == JT - 1))
                        step += 1
            cp2 = work.tile([128, R], f16, tag="cp2", bufs=1)
            nc.scalar.copy(cp2, ps_o2)
            z2c = work.tile([1, R], f16, tag="z2c", bufs=1)
            nc.vector.tensor_copy(z2c, ps_z2[0:1, :])
            rz2 = work.tile([1, R], f16, tag="rz", bufs=2)
            nc.vector.reciprocal(rz2, z2c)
            d_rz2 = dram.tile([1, R], f16, name="d_rz2")
            nc.sync.dma_start(out=d_rz2, in_=rz2)
            rz2b = work.tile([128, R], f16, tag="rz2b", bufs=1)
            nc.sync.dma_start(out=rz2b[:, None, :],
                              in_=d_rz2[0:1, :].partition_broadcast(128))
            o2 = work.tile([128, R], f16, tag="o2s", bufs=1)
            nc.vector.tensor_tensor(o2, cp2, rz2b, op=Al.mult)
            outT_sb = work.tile([OUT, R], f32, tag="outT", bufs=1)
            nc.scalar.activation(outT_sb, o2, A.Prelu,
                                 bias=b2_sb[:, 0:1], alpha=NEG_OUT)
            nc.sync.dma_start(out=d_outT[:, :], in_=outT_sb)

    nc.finalize()
    return nc


def _prep_host(x, adj, w1, att_src1, att_dst1, b1, w2, att_src2, att_dst2, b2):
    x = np.asarray(x, np.float32).reshape(N, F_IN)
    adj = np.asarray(adj, np.float32).reshape(N, N)
    w1 = np.asarray(w1, np.float32)
    w2 = np.asarray(w2, np.float32)
    att_src1 = np.asarray(att_src1, np.float32)
    att_dst1 = np.asarray(att_dst1, np.float32)
    att_src2 = np.asarray(att_src2, np.float32)
    att_dst2 = np.asarray(att_dst2, np.float32)
    b1 = np.asarray(b1, np.float32)
    b2 = np.asarray(b2, np.float32)

    xT16 = np.ascontiguousarray(x.T).astype(np.float16)
    adj16 = adj.astype(np.float16)
    v_src1 = np.empty((F_IN, H1), np.float32)
    v_dst1 = np.empty((F_IN, H1), np.float32)
    for h in range(H1):
        blk = w1[:, h * HID:(h + 1) * HID]
        v_src1[:, h] = blk @ att_src1[h]
        v_dst1[:, h] = blk @ att_dst1[h]
    rhs1 = np.concatenate([w1, v_src1], axis=1).astype(np.float16)
    v_src2 = (w2 @ att_src2[0])[:, None]
    v_dst2 = (w2 @ att_dst2[0])[:, None]
    rhs2 = np.concatenate([w2, v_src2, v_dst2], axis=1).astype(np.float16)
    b1c = np.ascontiguousarray(b1.reshape(H1, HID).T)
    b2c = np.ascontiguousarray(b2.reshape(OUT, 1))

    in_maps = []
    for c in range(N_CORES):
        rows = slice(c * R, (c + 1) * R)
        in_maps.append({
            "xT": xT16,
            "xmT": np.ascontiguousarray(xT16[:, rows]),
            "adjT": np.ascontiguousarray(adj16[rows, :].T),
            "rhs1": rhs1,
            "vdst1": v_dst1.astype(np.float16),
            "rhs2": rhs2,
            "b1c": b1c,
            "b2c": b2c,
        })
    return in_maps


def kernel(**inputs) -> np.ndarray:
    from concourse.bass_utils import run_bass_kernel_spmd

    if "nc" not in _CACHE:
        _CACHE["nc"] = _build()
    nc = _CACHE["nc"]
    in_maps = _prep_host(**inputs)
    try:
        res = run_bass_kernel_spmd(nc, in_maps, list(range(N_CORES)))
    except Exception:
        # transient NRT device wedge — one clean retry
        res = run_bass_kernel_spmd(nc, in_maps, list(range(N_CORES)))
    out = np.empty((1, N, OUT), np.float32)
    for c in range(N_CORES):
        out[0, c * R:(c + 1) * R, :] = res.results[c]["outT"].T
    return out


# revision 28
# speedup vs baseline: 1.0324x; 1.0324x over previous
"""Dense GAT (2-layer, 8+1 heads) on 8 Trainium2 NeuronCores — V3.

Row-parallel over destination rows (R=512 per core). Key algebra: softmax
normalization cancels any pure-i factor, so with s = a_src[j] + a_dst[i]:

    exp(lrelu(s))/exp(.2 ad[i]) = max(exp(as[j])*exp(.8 ad[i]), exp(.2 as[j]))

Per (head, j-tile) the unmasked weights are ONE 4x-mode DVE op:
    m = tensor_scalar(u8bc, scalar1=eas[j], scalar2=e2as[j], mult, max)
then masked with a quad-batched tensor_tensor against adjT. A fraction of
quads runs on ACT (Relu with AP scale/bias) + Pool (tensor_mul) to use all
three elementwise engines. adjT comes pre-transposed from the host; row
broadcasts go through a DRAM bounce (DMA, no engine time).
"""
import numpy as np

N = 4096
F_IN = 256
HID = 64
H1 = 8
F1 = H1 * HID
OUT = 128
N_CORES = 8
R = N // N_CORES
JT = N // 128
IT = R // 128
NQ = JT // 4           # jt quads per head
G2 = 68                # gather cols: 64 f32 = h2|ones fp16-packed, eas2, e2as2, ne2as2
NEG_ATT = 0.2
NEG_OUT = 0.01

# Quad engine assignment (Bresenham-interleaved fractions):
#   A = DVE op1 + DVE mask-mult
#   B = ACT(Relu)+ACT(Identity) + DVE mask-mult
#   D = DVE op1 + Pool mask-mult
QFRAC = {"B": 5, "D": 9, "A": 10}   # per 24 quads

_CACHE = {}
_QTYPES = []
_acc = {k: 0 for k in QFRAC}
for _g in range(80):
    for k in QFRAC:
        _acc[k] += QFRAC[k]
    _pick = max(_acc, key=lambda k: _acc[k])
    _acc[_pick] -= 24
    _QTYPES.append(_pick)


L2QTYPES = "ABADADBA"


def _qtype(g):
    if g >= H1 * NQ:
        return L2QTYPES[g - H1 * NQ]
    return _QTYPES[g]


def _build():
    import concourse.bass as bass
    from concourse import bacc
    import concourse.mybir as mybir
    import concourse.tile as tile
    from concourse.masks import make_identity

    f32 = mybir.dt.float32
    f16 = mybir.dt.float16
    A = mybir.ActivationFunctionType
    Al = mybir.AluOpType

    nc = bacc.Bacc("TRN2", target_bir_lowering=False, debug=False,
                   num_devices=N_CORES)
    d_xT = nc.dram_tensor("xT", [F_IN, N], f16, kind="ExternalInput")
    d_adjT = nc.dram_tensor("adjT", [N, R], f16, kind="ExternalInput")
    d_rhs1 = nc.dram_tensor("rhs1", [F_IN, F1 + H1], f16, kind="ExternalInput")
    d_vdst1 = nc.dram_tensor("vdst1", [F_IN, H1], f16, kind="ExternalInput")
    d_rhs2 = nc.dram_tensor("rhs2", [F1, OUT + 2], f16, kind="ExternalInput")
    d_b1c = nc.dram_tensor("b1c", [HID, H1], f32, kind="ExternalInput")
    d_b2c = nc.dram_tensor("b2c", [OUT, 1], f32, kind="ExternalInput")
    d_outT = nc.dram_tensor("outT", [OUT, R], f32, kind="ExternalOutput")
    c_lo = 0  # this core's own-row block: cols c*R..(c+1)*R of xT handled via xmT input
    d_xmT = nc.dram_tensor("xmT", [F_IN, R], f16, kind="ExternalInput")

    with tile.TileContext(nc) as tc, \
         nc.allow_low_precision(reason="fp16 softmax weights; tolerance 2e-2"):
        with tc.tile_pool(name="const", bufs=1) as const, \
             tc.tile_pool(name="big", bufs=1) as big, \
             tc.tile_pool(name="work", bufs=3) as work, \
             tc.tile_pool(name="dram", bufs=1, space="DRAM") as dram, \
             tc.tile_pool(name="ps_mm", bufs=2, space="PSUM") as ps_mm, \
             tc.tile_pool(name="ps_h2p", bufs=1, space="PSUM") as ps_h2p, \
             tc.tile_pool(name="ps_z2p", bufs=1, space="PSUM") as ps_z2p, \
             tc.tile_pool(name="ps_sm", bufs=2, space="PSUM") as ps_sm, \
             tc.tile_pool(name="ps_ag", bufs=2, space="PSUM") as ps_ag:
            ident = const.tile([128, 128], f32)
            make_identity(nc, ident)
            ones16 = const.tile([128, 1], f16)
            nc.vector.memset(ones16, 1.0)
            rhs1_sb = const.tile([128, 2, F1 + H1], f16)
            nc.sync.dma_start(
                out=rhs1_sb,
                in_=d_rhs1[:, :].rearrange("(kb p) f -> p kb f", p=128))
            vdst1_sb = const.tile([128, 2, H1], f16)
            nc.sync.dma_start(
                out=vdst1_sb,
                in_=d_vdst1[:, :].rearrange("(kb p) f -> p kb f", p=128))
            rhs2_sb = const.tile([128, 4, OUT + 2], f16)
            nc.sync.dma_start(
                out=rhs2_sb,
                in_=d_rhs2[:, :].rearrange("(kt p) f -> p kt f", p=128))
            b1_sb = const.tile([HID, H1], f32)
            nc.sync.dma_start(out=b1_sb, in_=d_b1c[:, :])
            b2_sb = const.tile([OUT, 1], f32)
            nc.sync.dma_start(out=b2_sb, in_=d_b2c[:, :])

            adjT_all = big.tile([128, JT, R], f16)          # 32 KB/part
            h1_all = big.tile([128, JT, H1, HID + 1], f16)  # 32.5 KB/part
            asrc_all = big.tile([128, JT, H1], f32)
            eas_all = big.tile([128, JT, H1], f32)
            e2as_all = big.tile([128, JT, H1], f32)
            ne2as_all = big.tile([128, JT, H1], f32)
            u8bc_all = big.tile([128, H1, R], f16)          # 8 KB/part
            u8bc2 = big.tile([128, R], f16)
            x2T_all = big.tile([128, 4, R], f16)
            adstT = big.tile([H1, R], f32)
            adst2T = big.tile([1, R], f32)
            h2g_all = big.tile([128, N_CORES, 4, G2], f32)  # 17.4 KB/part

            nc.vector.memset(h1_all[:, :, :, HID:HID + 1], 1.0)

            # ---- own rows first (tiny), then interleaved adjT/xT chunks ----
            xm16 = big.tile([128, 2, R], f16)
            nc.sync.dma_start(
                out=xm16, in_=d_xmT[:, :].rearrange("(kb p) r -> p kb r", p=128))
            xt_all = big.tile([128, 2, N], f16)           # 16 KB/part
            for b in range(4):
                nc.sync.dma_start(
                    out=adjT_all[:, b * 8:(b + 1) * 8, :],
                    in_=d_adjT[b * 1024:(b + 1) * 1024, :].rearrange(
                        "(jt p) r -> p jt r", p=128))
                cols = slice(b * 1024, (b + 1) * 1024)
                for kb in range(2):
                    nc.sync.dma_start(out=xt_all[:, kb, cols],
                                      in_=d_xT[kb * 128:(kb + 1) * 128, cols])

            # ---- a_dst (own rows) + u8 broadcasts ----
            for it in range(IT):
                ps_ad = ps_sm.tile([128, H1], f32, tag="sm")
                for kb in range(2):
                    nc.tensor.matmul(ps_ad,
                                     xm16[:, kb, it * 128:(it + 1) * 128],
                                     vdst1_sb[:, kb, :],
                                     start=(kb == 0), stop=(kb == 1))
                adm = work.tile([128, H1], f32, tag="adm", bufs=2)
                nc.vector.tensor_copy(adm, ps_ad)
                ps_adT = ps_sm.tile([H1, 128], f32, tag="sm")
                nc.tensor.transpose(ps_adT, adm, ident)
                nc.vector.tensor_copy(adstT[:, it * 128:(it + 1) * 128], ps_adT)
            u8rows = work.tile([H1, R], f16, tag="u8r", bufs=1)
            nc.scalar.activation(u8rows, adstT, A.Exp, scale=1.0 - NEG_ATT)
            d_u8 = dram.tile([H1, R], f16, name="d_u8")
            nc.sync.dma_start(out=d_u8, in_=u8rows)
            nc.sync.dma_start(out=u8bc_all[:, None, :, :],
                              in_=d_u8[:, :].partition_broadcast(128))

            # ---- h1 | a_src ----
            for jt in range(JT):
                cols = slice(jt * 128, (jt + 1) * 128)
                ps_h = ps_mm.tile([128, F1], f32, tag="h")
                ps_a = ps_sm.tile([128, H1], f32, tag="sm")
                for kb in range(2):
                    nc.tensor.matmul(ps_h, xt_all[:, kb, cols],
                                     rhs1_sb[:, kb, 0:F1],
                                     start=(kb == 0), stop=(kb == 1))
                    nc.tensor.matmul(ps_a, xt_all[:, kb, cols],
                                     rhs1_sb[:, kb, F1:F1 + H1],
                                     start=(kb == 0), stop=(kb == 1))
                h1dst = h1_all[:, jt, :, 0:HID]
                h1src = ps_h.rearrange("p (h c) -> p h c", c=HID)
                if jt % 4 == 0 and jt < 16:
                    nc.vector.tensor_copy(h1dst, h1src)
                else:
                    nc.scalar.copy(h1dst, h1src)
                nc.scalar.copy(asrc_all[:, jt, :], ps_a)
                if jt % 8 == 7:
                    g = slice(jt - 7, jt + 1)
                    nc.scalar.activation(eas_all[:, g, :],
                                         asrc_all[:, g, :], A.Exp)
                    nc.scalar.activation(e2as_all[:, g, :],
                                         asrc_all[:, g, :], A.Exp,
                                         scale=NEG_ATT)
                    nc.vector.tensor_scalar_mul(ne2as_all[:, g, :],
                                                e2as_all[:, g, :], -1.0)

            ps_h2m = ps_h2p.tile([128, 4, OUT], f32, name="ph2m")  # 1 bank

            # PE pacing: tiny matmuls dependent on freshly produced data keep
            # the tensor engine's p-state run alive through producer stalls.
            def pe_tick(src_slice):
                ps_d = ps_sm.tile([64, 64], f32, tag="sm")
                nc.tensor.matmul(ps_d, src_slice, src_slice[:, 0:64],
                                 start=True, stop=True)

            # ---- layer-1 attention ----
            bounce_in = dram.tile([R, G2], f32, name="bin")
            bounce_out = dram.tile([N_CORES, R, G2], f32,
                                   addr_space="Shared", name="bout")
            for h in range(H1):
                ps_agg = ps_ag.tile([HID + 1, R], f32, tag="agg")
                for q in range(NQ):
                    qt = _qtype(h * NQ + q)
                    jts = [q * 4 + k for k in range(4)]
                    adjq = adjT_all[:, q * 4:q * 4 + 4, :]
                    e_q = work.tile([128, 4, R], f16, tag="eq", bufs=5)
                    m_q = work.tile([128, 4, R], f16, tag="mq", bufs=5)
                    if qt == "B":
                        for k, jt in enumerate(jts):
                            t_q = work.tile([128, R], f16, tag="tq", bufs=4)
                            nc.scalar.activation(
                                t_q, u8bc_all[:, h, :], A.Relu,
                                bias=ne2as_all[:, jt, h:h + 1],
                                scale=eas_all[:, jt, h:h + 1])
                            nc.scalar.activation(
                                m_q[:, k, :], t_q, A.Identity,
                                bias=e2as_all[:, jt, h:h + 1])
                            pe_tick(m_q[:, k, 0:64])
                    else:
                        for k, jt in enumerate(jts):
                            nc.vector.tensor_scalar(
                                m_q[:, k, :], u8bc_all[:, h, :],
                                eas_all[:, jt, h:h + 1],
                                e2as_all[:, jt, h:h + 1],
                                op0=Al.mult, op1=Al.max)
                            pe_tick(m_q[:, k, 0:64])
                    if qt == "D":
                        nc.gpsimd.tensor_mul(e_q, m_q, adjq)
                    elif qt == "B":
                        for z in range(2):
                            nc.vector.tensor_tensor(
                                e_q[:, z * 2:z * 2 + 2, :],
                                m_q[:, z * 2:z * 2 + 2, :],
                                adjq[:, z * 2:z * 2 + 2, :], op=Al.mult)
                    else:
                        nc.vector.tensor_tensor(e_q, m_q, adjq, op=Al.mult)
                    for k, jt in enumerate(jts):
                        nc.tensor.matmul(ps_agg, h1_all[:, jt, h, :],
                                         e_q[:, k, :],
                                         start=(jt == 0), stop=(jt == JT - 1))
                # fast-release: one ACT copy frees ps_agg, then normalize
                # runs on SBUF fp16 off the critical path
                cp = work.tile([HID + 1, R], f16, tag="cp", bufs=3)
                nc.vector.tensor_copy(cp, ps_agg)
                rz = work.tile([1, R], f16, tag="rz", bufs=2)
                nc.vector.reciprocal(rz, cp[HID:HID + 1, :])
                d_rz = dram.tile([1, R], f16, name=f"d_rz{h}")
                nc.sync.dma_start(out=d_rz, in_=rz)
                rzb = work.tile([HID, R], f16, tag="rzb", bufs=2)
                nc.sync.dma_start(out=rzb[:, None, :],
                                  in_=d_rz[0:1, :].partition_broadcast(HID))
                y_h = work.tile([HID, R], f16, tag="yh", bufs=2)
                nc.vector.tensor_tensor(y_h, cp[0:HID, :], rzb, op=Al.mult)
                po = (h % 2) * HID
                nc.scalar.activation(
                    x2T_all[po:po + HID, h // 2, :], y_h, A.Prelu,
                    bias=b1_sb[:, h:h + 1], alpha=NEG_OUT)

            # ---- layer 2: pack h2 + gather cols, single AllGather ----
            for it in range(IT):
                cols = slice(it * 128, (it + 1) * 128)
                ps_a2 = ps_sm.tile([128, 2], f32, tag="sm")
                for kt in range(4):
                    nc.tensor.matmul(ps_a2, x2T_all[:, kt, cols],
                                     rhs2_sb[:, kt, OUT:OUT + 2],
                                     start=(kt == 0), stop=(kt == 3))
                for kt in range(4):
                    nc.tensor.matmul(ps_h2m[:, it, :],
                                     x2T_all[:, kt, cols],
                                     rhs2_sb[:, kt, 0:OUT],
                                     start=(kt == 0), stop=(kt == 3))
                h2m = work.tile([128, G2], f32, tag="h2m", bufs=2)
                h2m16 = h2m.bitcast(f16)
                nc.scalar.copy(h2m16[:, 0:OUT], ps_h2m[:, it, :])
                nc.vector.memset(h2m16[:, OUT:OUT + 2], 1.0)
                nc.scalar.activation(h2m[:, 65:66], ps_a2[:, 0:1], A.Exp)
                nc.scalar.activation(h2m[:, 66:67], ps_a2[:, 0:1],
                                     A.Exp, scale=NEG_ATT)
                nc.vector.tensor_scalar_mul(h2m[:, 67:68], h2m[:, 66:67], -1.0)
                nc.sync.dma_start(
                    out=bounce_in[it * 128:(it + 1) * 128, :], in_=h2m)
                ad2c = work.tile([128, 1], f32, tag="ad2c", bufs=2)
                nc.vector.tensor_copy(ad2c, ps_a2[:, 1:2])
                ps_adT2 = ps_sm.tile([1, 128], f32, tag="sm")
                nc.tensor.transpose(ps_adT2, ad2c, ident)
                nc.vector.tensor_copy(adst2T[:, it * 128:(it + 1) * 128],
                                      ps_adT2)
            nc.gpsimd.collective_compute(
                "AllGather",
                bass.mybir.AluOpType.bypass,
                replica_groups=[list(range(N_CORES))],
                ins=[bounce_in.opt()],
                outs=[bounce_out.opt()],
            )
            u8row2 = work.tile([1, R], f16, tag="u8r2", bufs=1)
            nc.scalar.activation(u8row2, adst2T, A.Exp, scale=1.0 - NEG_ATT)
            d_u82 = dram.tile([1, R], f16, name="d_u82")
            nc.sync.dma_start(out=d_u82, in_=u8row2)
            nc.sync.dma_start(out=u8bc2[:, None, :],
                              in_=d_u82[0:1, :].partition_broadcast(128))
            for c4 in range(N_CORES):
                nc.sync.dma_start(
                    out=h2g_all[:, c4, :, :],
                    in_=bounce_out[c4].rearrange("(r p) g -> p r g", p=128))

            # ---- layer-2 attention ----
            ps_o2 = ps_ag.tile([128, R], f32, tag="agg")
            ps_z2 = ps_z2p.tile([1, R], f32, name="z2")
            L2P = "ADBADADAADBADADA"

            def slot(jt):
                return h2g_all[:, jt // 4, jt % 4, :]

            step = 0
            for pr in range(JT // 2):
                    jts = [pr * 2, pr * 2 + 1]
                    qt = L2P[pr % len(L2P)]
                    adjp = adjT_all[:, jts[0]:jts[0] + 2, :]
                    e_q = work.tile([128, 2, R], f16, tag="eq2", bufs=5)
                    m_q = work.tile([128, 2, R], f16, tag="mq2", bufs=5)
                    if qt == "B":
                        for k, jt in enumerate(jts):
                            h2g = slot(jt)
                            t_q = work.tile([128, R], f16, tag="tq", bufs=4)
                            nc.scalar.activation(
                                t_q, u8bc2, A.Relu,
                                bias=h2g[:, 67:68], scale=h2g[:, 65:66])
                            nc.scalar.activation(
                                m_q[:, k, :], t_q, A.Identity,
                                bias=h2g[:, 66:67])
                    else:
                        for k, jt in enumerate(jts):
                            h2g = slot(jt)
                            nc.vector.tensor_scalar(
                                m_q[:, k, :], u8bc2,
                                h2g[:, 65:66], h2g[:, 66:67],
                                op0=Al.mult, op1=Al.max)
                            pe_tick(m_q[:, k, 0:64])
                    if qt == "D":
                        nc.gpsimd.tensor_mul(e_q, m_q, adjp)
                    else:
                        nc.vector.tensor_tensor(e_q, m_q, adjp, op=Al.mult)
                    for k, jt in enumerate(jts):
                        h2g = slot(jt)
                        h2g16 = h2g[:, 0:OUT // 2].bitcast(f16)
                        ones16 = h2g[:, OUT // 2:OUT // 2 + 1].bitcast(f16)[:, 0:1]
                        nc.tensor.matmul(ps_o2, h2g16, e_q[:, k, :],
                                         start=(step == 0), stop=(step == JT - 1))
                        nc.tensor.matmul(ps_z2, ones16, e_q[:, k, :],
                                         start=(step == 0), stop=(step == JT - 1))
                        step += 1
            cp2 = work.tile([128, R], f16, tag="cp2", bufs=1)
            nc.scalar.copy(cp2, ps_o2)
            z2c = work.tile([1, R], f16, tag="z2c", bufs=1)
            nc.vector.tensor_copy(z2c, ps_z2[0:1, :])
            rz2 = work.tile([1, R], f16, tag="rz", bufs=2)
            nc.vector.reciprocal(rz2, z2c)
            d_rz2 = dram.tile([1, R], f16, name="d_rz2")
            nc.sync.dma_start(out=d_rz2, in_=rz2)
            rz2b = work.tile([128, R], f16, tag="rz2b", bufs=1)
            nc.sync.dma_start(out=rz2b[:, None, :],
                              in_=d_rz2[0:1, :].partition_broadcast(128))
            o2 = work.tile([128, R], f16, tag="o2s", bufs=1)
            nc.vector.tensor_tensor(o2, cp2, rz2b, op=Al.mult)
            outT_sb = work.tile([OUT, R], f32, tag="outT", bufs=1)
            nc.scalar.activation(outT_sb, o2, A.Prelu,
                                 bias=b2_sb[:, 0:1], alpha=NEG_OUT)
            nc.sync.dma_start(out=d_outT[:, :], in_=outT_sb)

    nc.finalize()
    return nc


def _prep_host(x, adj, w1, att_src1, att_dst1, b1, w2, att_src2, att_dst2, b2):
    x = np.asarray(x, np.float32).reshape(N, F_IN)
    adj = np.asarray(adj, np.float32).reshape(N, N)
    w1 = np.asarray(w1, np.float32)
    w2 = np.asarray(w2, np.float32)
    att_src1 = np.asarray(att_src1, np.float32)
    att_dst1 = np.asarray(att_dst1, np.float32)
    att_src2 = np.asarray(att_src2, np.float32)
    att_dst2 = np.asarray(att_dst2, np.float32)
    b1 = np.asarray(b1, np.float32)
    b2 = np.asarray(b2, np.float32)

    xT16 = np.ascontiguousarray(x.T).astype(np.float16)
    adj16 = adj.astype(np.float16)
    v_src1 = np.empty((F_IN, H1), np.float32)
    v_dst1 = np.empty((F_IN, H1), np.float32)
    for h in range(H1):
        blk = w1[:, h * HID:(h + 1) * HID]
        v_src1[:, h] = blk @ att_src1[h]
        v_dst1[:, h] = blk @ att_dst1[h]
    rhs1 = np.concatenate([w1, v_src1], axis=1).astype(np.float16)
    v_src2 = (w2 @ att_src2[0])[:, None]
    v_dst2 = (w2 @ att_dst2[0])[:, None]
    rhs2 = np.concatenate([w2, v_src2, v_dst2], axis=1).astype(np.float16)
    b1c = np.ascontiguousarray(b1.reshape(H1, HID).T)
    b2c = np.ascontiguousarray(b2.reshape(OUT, 1))

    in_maps = []
    for c in range(N_CORES):
        rows = slice(c * R, (c + 1) * R)
        in_maps.append({
            "xT": xT16,
            "xmT": np.ascontiguousarray(xT16[:, rows]),
            "adjT": np.ascontiguousarray(adj16[rows, :].T),
            "rhs1": rhs1,
            "vdst1": v_dst1.astype(np.float16),
            "rhs2": rhs2,
            "b1c": b1c,
            "b2c": b2c,
        })
    return in_maps


def kernel(**inputs) -> np.ndarray:
    from concourse.bass_utils import run_bass_kernel_spmd

    if "nc" not in _CACHE:
        _CACHE["nc"] = _build()
    nc = _CACHE["nc"]
    in_maps = _prep_host(**inputs)
    try:
        res = run_bass_kernel_spmd(nc, in_maps, list(range(N_CORES)))
    except Exception:
        # transient NRT device wedge — one clean retry
        res = run_bass_kernel_spmd(nc, in_maps, list(range(N_CORES)))
    out = np.empty((1, N, OUT), np.float32)
    for c in range(N_CORES):
        out[0, c * R:(c + 1) * R, :] = res.results[c]["outT"].T
    return out
